# revision 23
# baseline (speedup 1.0000x reference)
"""Multi-head attention (B=2, S=2048, D=1024, H=16) on 8 Trainium2 cores.

Sharding: core = (batch b in {0,1}) x (head-group g in {0..3}).
Each core computes, for its batch, 4 heads over the full sequence:
  - Q^T, K^T projections (pair-packed [dk128, 2, S]) and V (natural,
    with an appended ones-column) from host-pre-transposed X^T inputs,
  - scores computed transposed S^T[k, q] per (q-chunk 512, head-pair):
    two row-packed matmuls (tile_position rows 0/64) that overlap on
    the PE for an effective 2x rate on the dk=64 contraction,
  - exp on the scalar/Act engine (the kernel's throughput floor),
  - PV in O-natural form: O[q, dk+1] = sum_kb P^T[kb,qblock].T @ [V|1]
    -- streams 65 columns per (head, q-block, kb) with the [128,128]
    stationary load hidden, ~2x faster than the O^T form,
  - per-(q,head) normalization via DVE reciprocal + DVE scale (the
    Pool/gpsimd engine cannot touch PSUM),
  - PE transposes (vs identity) rebuild O^T, then the output
    projection accumulates over head-pairs; partials leave as bf16.
Host sums the 4 partials per batch and adds bo.

PSUM notes: matmul start=True resets the addressed partitions' whole
2KB bank, so the 8 PV accumulators packed into one 2-bank tile use
start=False over a memset-zeroed region, and each transpose q-block
gets its own bank-padded tile.

Emission is software-pipelined: each (qc, pair) pass emits only its
scores+exp inline; its PV/normalize/outproj work is deferred into a
later pass's kb loop (popped between score tiles). Act-engine relief:
FAST_KB slots run exp on the DVE as an int16 Schraudolph whose result
bits ARE the bf16 exp (each slot adds ~0.5%*sqrt(n/16) output noise).
Non-final normalizations hand their O->O^T transposes to the DMA xbar
(dma_start_transpose) instead of the PE. Outproj closures drain lazily
in passes 4+ where the schedule has no projection fillers, keeping the
PE dense enough to hold the HAM clock at 2.4GHz. Startup: K-path DMAs
on the sync HWDGE queue and Q-path on the scalar queue in parallel,
with junk matmuls + a dummy exp pre-warming the PE clock and Act
tables during the DMA wait.
"""

import ml_dtypes
import numpy as np

import concourse.bass as bass
import concourse.bacc as bacc
import concourse.mybir as mybir
import concourse.tile as tile
from concourse.bass_utils import run_bass_kernel_spmd

F32 = mybir.dt.float32
BF16 = mybir.dt.bfloat16
AF = mybir.ActivationFunctionType

B = 2
S = 2048
D = 1024
H = 16
DK = 64
GH = 4            # heads per core
GD = GH * DK      # 256: projection slice width per core
SC = 256          # s-chunk for projections
NSC = S // SC     # 8
NDC = D // 128    # 8 contraction chunks
QC = 512          # q-chunk for attention
NQC = S // QC     # 4
NKB = S // 128    # 16 key blocks
SCALE = 1.0 / np.sqrt(np.float32(DK))
# scores arrive pre-scaled (SCALE folded into Wq/bq on the host)
FE_A = float(np.float32(2 ** 7 / np.log(2.0)))
FE_B = float(np.float32(127 * 2 ** 7 - 486411.0 / 2 ** 16))
I16 = mybir.dt.int16
# kb slots whose exp runs on the DVE via int16-Schraudolph (bf16 bit
# pattern built directly by mult+add -> int16 convert). Each fast slot
# adds ~exp noise |ds|<=0.04 to 1/16 of the keys; rel-err grows
# ~sqrt(n_fast/16)*2e-2, so keep this small.
FAST_KB = (3, 8, 13)


def build_nc():
    nc = bacc.Bacc()

    xqt = nc.dram_tensor("xqt", [NSC, 128, NDC, SC], BF16, kind="ExternalInput")
    xkt = nc.dram_tensor("xkt", [NSC, 128, NDC, SC], BF16, kind="ExternalInput")
    xvt = nc.dram_tensor("xvt", [NSC, 128, NDC, SC], BF16, kind="ExternalInput")
    wq = nc.dram_tensor("wq", [128, 2, NDC, 128], BF16, kind="ExternalInput")
    wk = nc.dram_tensor("wk", [128, 2, NDC, 128], BF16, kind="ExternalInput")
    wv = nc.dram_tensor("wv", [128, NDC, GD], BF16, kind="ExternalInput")
    wo = nc.dram_tensor("wo", [128, 2, D], BF16, kind="ExternalInput")
    bq = nc.dram_tensor("bq", [GD], F32, kind="ExternalInput")
    bk = nc.dram_tensor("bk", [GD], F32, kind="ExternalInput")
    bv = nc.dram_tensor("bv", [GD], F32, kind="ExternalInput")
    idn = nc.dram_tensor("idn", [128, 128], BF16, kind="ExternalInput")
    out = nc.dram_tensor("out", [S, D], BF16, kind="ExternalOutput")

    with tile.TileContext(nc) as tc:
        with (
            tc.tile_pool(name="persist", bufs=1) as persist,
            tc.tile_pool(name="stage", bufs=5) as stage,
            tc.tile_pool(name="ptp", bufs=36) as ptp,
            tc.tile_pool(name="onp", bufs=2) as onp,
            tc.tile_pool(name="otp", bufs=3) as otp,
            tc.tile_pool(name="obp", bufs=2) as obp,
            tc.tile_pool(name="work", bufs=2) as work,
            tc.tile_pool(name="pst", bufs=2, space="PSUM") as pst,
            tc.tile_pool(name="pmix", bufs=1, space="PSUM") as pmix,
            tc.tile_pool(name="pop", bufs=2, space="PSUM") as pop,
        ):
            # ---- persistent tiles -----------------------------------------
            wk_sb = persist.tile([128, 2, NDC, 128], BF16, tag="wk_sb")
            wq_sb = persist.tile([128, 2, NDC, 128], BF16, tag="wq_sb")
            wv_sb = persist.tile([128, NDC, GD], BF16, tag="wv_sb")
            wo_sb = persist.tile([128, 2, D], BF16, tag="wo_sb")
            bq_sb = persist.tile([128, 2], F32, tag="bq_sb")
            bk_sb = persist.tile([128, 2], F32, tag="bk_sb")
            bv_bcast = persist.tile([128, GD], F32, tag="bv_bcast")
            idn_sb = persist.tile([128, 128], BF16, tag="idn_sb")

            qt_sb = persist.tile([128, 2, S], BF16, tag="qt_sb")
            kt_sb = persist.tile([128, 2, S], BF16, tag="kt_sb")
            vhat_sb = persist.tile([128, NKB, GH, DK + 1], BF16, tag="vhat_sb")

            xk_tiles = {}
            xq_tiles = {}
            xv_tiles = {}

            def dma_x(which, sc, eng=None):
                t = stage.tile([128, NDC, SC], BF16, tag=f"x{which}_t")
                src = {"q": xqt, "k": xkt, "v": xvt}[which]
                (eng or nc.sync).dma_start(out=t, in_=src[sc])
                return t

            # ---- startup DMAs: wk0 leads the sync queue so the PE warmup
            # can start ~2us in; K path on sync, Q path on the scalar HWDGE
            # queue (queues share the 16 SDMA engines, but splitting avoids
            # FIFO head-of-line blocking between the two critical paths).
            nc.sync.dma_start(out=bk_sb,
                              in_=bk[:].rearrange("(c p) -> p c", p=128))
            nc.sync.dma_start(out=wk_sb[:, 0], in_=wk[:, 0])
            xk_tiles[0] = dma_x("k", 0)
            nc.scalar.dma_start(out=wq_sb[:, 0], in_=wq[:, 0])
            xq_tiles[0] = dma_x("q", 0, nc.scalar)
            xq_tiles[1] = dma_x("q", 1, nc.scalar)
            # PE warmup: junk matmuls on the first-arrived weights keep the
            # PE busy through the HAM activity window during the DMA wait,
            # so the first real projections run at 2.4 GHz, not 1.2.
            jk = pst.tile([128, 2 * QC], F32, tag="st")
            for j in range(8):
                nc.tensor.matmul(
                    jk[:, bass.ts(j % 2, 512)], lhsT=wk_sb[:, 0, 0, :],
                    rhs=wk_sb[:, 0, 0:4].rearrange("p a b -> p (a b)"),
                    start=True, stop=True, skip_group_check=True,
                )
            # Act table preload: a dummy exp right after the (tiny) bk DMA
            # pulls the ~2.7us exp table-set load off the first-score path.
            warm = work.tile([128, 1], F32, tag="warm")
            nc.scalar.activation(warm, bk_sb[:, 0:1], AF.Exp, scale=1.0)
            nc.scalar.dma_start(out=bq_sb,
                                in_=bq[:].rearrange("(c p) -> p c", p=128))
            nc.sync.dma_start(out=wk_sb[:, 1], in_=wk[:, 1])
            xk_tiles[1] = dma_x("k", 1)
            nc.scalar.dma_start(out=wq_sb[:, 1], in_=wq[:, 1])
            bv_ap = bv[:]
            nc.gpsimd.dma_start(
                out=bv_bcast,
                in_=bass.AP(tensor=bv_ap.tensor, offset=bv_ap.offset,
                            ap=[[0, 128]] + [list(p) for p in bv_ap.ap]),
            )
            nc.vector.memset(vhat_sb[:, :, :, DK:DK + 1], 1.0)

            # ---- projection emitters (half-chunk units, ~0.9us each) ------
            def proj_qk(which, sc, c):
                ss = bass.ts(sc, SC)
                if which == "k":
                    x_t, w_sb, b_sb, dst = xk_tiles[sc], wk_sb, bk_sb, kt_sb
                else:
                    x_t, w_sb, b_sb, dst = xq_tiles[sc], wq_sb, bq_sb, qt_sb
                ps = pop.tile([128, SC], F32, tag="op")
                for dc in range(NDC):
                    nc.tensor.matmul(
                        ps,
                        lhsT=w_sb[:, c, dc, :],
                        rhs=x_t[:, dc, :],
                        start=(dc == 0), stop=(dc == NDC - 1),
                    )
                nc.vector.tensor_scalar_add(
                    out=dst[:, c, ss], in0=ps, scalar1=b_sb[:, c:c + 1]
                )

            def proj_v(sc, half):
                x_t = xv_tiles[sc]
                kb = sc * (SC // 128) + half
                ps = pop.tile([128, GD], F32, tag="op")
                for dc in range(NDC):
                    nc.tensor.matmul(
                        ps,
                        lhsT=x_t[:, dc, bass.ts(half, 128)],
                        rhs=wv_sb[:, dc, :],
                        start=(dc == 0), stop=(dc == NDC - 1),
                    )
                nc.vector.tensor_add(
                    out=vhat_sb[:, kb, :, 0:DK],
                    in0=ps.rearrange("p (h d) -> p h d", h=GH),
                    in1=bv_bcast.rearrange("p (h d) -> p h d", h=GH),
                )

            # per-pass unit schedule: dma units are cheap (queue push) and
            # ride along; compute units are ~0.9us, at most one per kb slot.
            # q-pair units sit 1-2 passes before their consuming pass so the
            # late passes keep enough PE filler work to hold HAM at 8/8.
            sched = [[] for _ in range(2 * NQC)]
            sched[0] = [
                ("k", 0, 1), ("dk", 2, 0), ("k", 1, 0), ("k", 1, 1),
                ("dk", 3, 0), ("k", 2, 0), ("k", 2, 1),
                ("dk", 4, 0), ("k", 3, 0), ("k", 3, 1),
                ("dwv", 0, 0), ("dk", 5, 0), ("k", 4, 0), ("k", 4, 1),
                ("dk", 6, 0), ("k", 5, 0), ("dv", 0, 0), ("k", 5, 1),
                ("dk", 7, 0), ("k", 6, 0), ("dv", 1, 0), ("q", 0, 1),
                ("k", 7, 0), ("dq", 2, 0), ("q", 1, 1), ("dv", 2, 0),
                ("k", 6, 1),
            ]
            sched[1] = [
                ("k", 7, 1), ("q", 2, 0), ("dv", 3, 0),
                ("v", 0, 0), ("dq", 3, 0), ("v", 0, 1),
                ("v", 1, 0), ("dv", 4, 0), ("v", 1, 1),
                ("q", 3, 0),
                ("v", 2, 0), ("dv", 5, 0), ("v", 2, 1),
                ("v", 3, 0), ("dv", 6, 0), ("v", 3, 1),
                ("v", 4, 0), ("dv", 7, 0), ("v", 4, 1),
            ]
            sched[2] = [
                ("v", 5, 0), ("v", 5, 1), ("v", 6, 0), ("v", 6, 1),
                ("v", 7, 0), ("dq", 4, 0), ("v", 7, 1),
                ("q", 2, 1), ("dwo", 0, 0), ("q", 3, 1),
            ]
            sched[3] = [("q", 4, 0), ("dq", 5, 0), ("q", 5, 0)]
            sched[4] = [("q", 4, 1), ("dq", 6, 0), ("di", 0, 0), ("q", 5, 1)]
            sched[5] = [("q", 6, 0), ("dq", 7, 0), ("q", 7, 0)]
            sched[6] = [("q", 6, 1), ("q", 7, 1)]

            def emit_units(units, n=1):
                for _ in range(n):
                    while units and units[0][0].startswith("d"):
                        kind, sc, part = units.pop(0)
                        if kind == "dk":
                            xk_tiles[sc] = dma_x("k", sc)
                        elif kind == "dq":
                            xq_tiles[sc] = dma_x("q", sc, nc.scalar)
                        elif kind == "dv":
                            xv_tiles[sc] = dma_x("v", sc)
                        elif kind == "dwv":
                            nc.sync.dma_start(out=wv_sb, in_=wv[:, :, :])
                        elif kind == "dwo":
                            nc.scalar.dma_start(out=wo_sb, in_=wo[:, :, :])
                        elif kind == "di":
                            nc.sync.dma_start(out=idn_sb, in_=idn[:, :])
                    if not units:
                        return
                    kind, sc, part = units.pop(0)
                    if kind == "k":
                        proj_qk("k", sc, part)
                    elif kind == "q":
                        proj_qk("q", sc, part)
                    elif kind == "v":
                        proj_v(sc, part)

            # K sc0 + Q sc0,sc1 pair-0 halves up front; pair-1 halves are
            # scheduled units (only needed once pass 1 starts)
            proj_qk("k", 0, 0)
            proj_qk("q", 0, 0)
            proj_qk("q", 1, 0)

            # ---- attention, software-pipelined ----------------------------
            post = []   # FIFO of deferred closures (PV / norm / outproj)

            def emit_scores_exp(qc, p, kb, fast=False):
                qs = bass.ts(qc, QC)
                ks = bass.ts(kb, 128)
                st = pst.tile([128, 2 * QC], F32, tag="st")
                nc.tensor.matmul(
                    st[:, 0:QC], lhsT=kt_sb[0:64, p, ks],
                    rhs=qt_sb[0:64, p, qs], start=True, stop=True,
                )
                nc.tensor.matmul(
                    st[:, QC:2 * QC], lhsT=kt_sb[64:128, p, ks],
                    rhs=qt_sb[64:128, p, qs], start=True, stop=True,
                    tile_position=(64, 0),
                )
                ti = ptp.tile([128, 2 * QC], I16, tag="pt")
                pt = ti.bitcast(BF16)
                if fast:
                    # Schraudolph exp on DVE: int16 result IS the bf16
                    # bit pattern of exp(st) (~1.5% rms rel err)
                    nc.vector.tensor_scalar(
                        out=ti, in0=st, scalar1=FE_A, scalar2=FE_B,
                        op0=mybir.AluOpType.mult, op1=mybir.AluOpType.add,
                    )
                else:
                    nc.scalar.activation(pt, st, AF.Exp, scale=1.0)
                return pt

            def pv_closure(p, kb, pt, mix):
                # kb==0: the first matmul into each 2KB PSUM bank (s=0
                # and s=4) uses start=True, which resets the whole bank;
                # the other 3 accumulators in that bank then see
                # has_written=0 and overwrite. Replaces a DVE memset.
                def emit():
                    for hl in range(2):
                        for qb in range(4):
                            s = hl * 4 + qb
                            nc.tensor.matmul(
                                mix[:, s, 0:DK + 1],
                                lhsT=pt[:, hl * QC + qb * 128:
                                        hl * QC + qb * 128 + 128],
                                rhs=vhat_sb[:, kb, 2 * p + hl, :],
                                start=(kb == 0 and s % 4 == 0),
                                stop=(kb == NKB - 1),
                                skip_group_check=True,
                            )
                return emit

            def norm_closure(p, mix, ot):
                # onat laid out [128, qb, hl, 64] so each qb slice is a
                # contiguous [128,128] that the DMA xbar can transpose
                # straight into ot (off the PE).
                def emit():
                    nonlocal norms_emitted
                    norms_emitted += 1
                    rs = work.tile([128, 8], F32, tag="rs")
                    nc.vector.reciprocal(
                        out=rs,
                        in_=mix[:, :, DK:DK + 1].rearrange("p a b -> p (a b)"),
                    )
                    onat = onp.tile([128, 4, 2, DK], BF16, tag="onat")
                    for s in range(8):
                        hl, qb = divmod(s, 4)
                        nc.vector.tensor_scalar_mul(
                            out=onat[:, qb, hl, :], in0=mix[:, s, 0:DK],
                            scalar1=rs[:, s:s + 1],
                        )
                    for qb in range(4):
                        nc.sync.dma_start_transpose(
                            out=ot[:, p * 4 + qb, :], in_=onat[:, qb])
                return emit

            def outproj_dm_closure(qc, qb, dm, ot, obuf, last=False):
                def emit():
                    op = pop.tile([128, 512], F32, tag="op")
                    for p in range(2):
                        nc.tensor.matmul(
                            op,
                            lhsT=ot[:, p * 4 + qb, :],
                            rhs=wo_sb[:, p, bass.ts(dm, 512)],
                            start=(p == 0), stop=(p == 1),
                        )
                    if (qb + dm) % 2:
                        nc.scalar.copy(obuf[:, bass.ts(dm, 512)], op)
                    else:
                        nc.vector.tensor_copy(obuf[:, bass.ts(dm, 512)], op)
                    if dm == 1:
                        r0 = qc * QC + qb * 128
                        nc.sync.dma_start(out=out[r0:r0 + 128, :], in_=obuf)
                return emit

            def outproj_closures(qc, qb, ot, last=False):
                obuf = obp.tile([128, D], BF16, tag="obuf")
                return [outproj_dm_closure(qc, qb, dm, ot, obuf, last)
                        for dm in range(2)]

            lazy = []   # deferred outproj closures, as (need_norms, fn):
                        # drained in later passes to keep the (filler-poor)
                        # back half PE-dense. A group may only be EMITTED
                        # once both of its qc's norms have been emitted --
                        # earlier emission would order the outproj's ot read
                        # BEFORE the norm's DMA-transpose write, so the
                        # framework would never see the dependency (races).
            norms_emitted = 0
            for qc in range(NQC):
                ot = otp.tile([128, 8, 128], BF16, tag="ot")
                for p in range(2):
                    pidx = 2 * qc + p
                    last = (pidx == 2 * NQC - 1)
                    units = sched[pidx]
                    mix = pmix.tile([128, 8, 128], F32, tag="mix")
                    own_pv = []
                    cooldown = 0
                    for kb in range(NKB):
                        ncu = sum(1 for u in units if not u[0].startswith("d"))
                        emit_units(units, 2 if ncu > NKB - kb else 1)
                        if pidx <= 1:
                            npop = 0
                        elif last or pidx >= 6 or len(post) > 32:
                            npop = 3
                        elif len(post) > 16:
                            npop = 2
                        else:
                            npop = 1
                        if cooldown > 0 and not last:
                            cooldown -= 1
                            npop = 0
                        for _ in range(npop):
                            if not post:
                                break
                            kind, fn = post.pop(0)
                            fn()
                            if kind == "norm" and not last:
                                cooldown = 1
                                break
                        for _ in range(2):
                            if lazy and norms_emitted >= lazy[0][0]:
                                lazy.pop(0)[1]()
                        if last and kb >= 6:
                            for _ in range(2):
                                if own_pv:
                                    own_pv.pop(0)()
                        pt = emit_scores_exp(qc, p, kb, fast=kb in FAST_KB)
                        own_pv.append(pv_closure(p, kb, pt, mix))
                    if last:
                        while own_pv:
                            own_pv.pop(0)()
                        while lazy:
                            lazy.pop(0)[1]()
                        # fused tail: normalize, transpose and project one
                        # q-block at a time so PE/DVE/Act pipeline the drain
                        rs = work.tile([128, 8], F32, tag="rs")
                        nc.vector.reciprocal(
                            out=rs, in_=mix[:, :, DK:DK + 1]
                            .rearrange("p a b -> p (a b)"),
                        )
                        onat = onp.tile([128, 4, 2, DK], BF16, tag="onat")
                        for qb in range(4):
                            for hl in range(2):
                                s = hl * 4 + qb
                                if hl:
                                    nc.scalar.mul(onat[:, qb, hl, :],
                                                  mix[:, s, 0:DK],
                                                  rs[:, s:s + 1])
                                else:
                                    nc.vector.tensor_scalar_mul(
                                        out=onat[:, qb, hl, :],
                                        in0=mix[:, s, 0:DK],
                                        scalar1=rs[:, s:s + 1],
                                    )
                            tp = pop.tile([128, 128], BF16, tag="op")
                            for hl in range(2):
                                nc.tensor.transpose(
                                    tp[64 * hl:64 * hl + 64, :],
                                    onat[:, qb, hl, :], idn_sb,
                                )
                            if qb % 2:
                                nc.scalar.copy(ot[:, p * 4 + qb, :], tp)
                            else:
                                nc.vector.tensor_copy(ot[:, p * 4 + qb, :], tp)
                            for c in outproj_closures(qc, qb, ot, last=True):
                                c()
                    else:
                        post += [("pv", c) for c in own_pv]
                        post.append(("norm", norm_closure(p, mix, ot)))
                if qc < NQC - 1:
                    for qb in range(4):
                        lazy += [(2 * qc + 2, c)
                                 for c in outproj_closures(qc, qb, ot)]

            while post:
                post.pop(0)[1]()
    return nc


_NC_CACHE = None


def _get_nc():
    global _NC_CACHE
    if _NC_CACHE is None:
        nc = build_nc()
        nc.finalize()
        _NC_CACHE = nc
    return _NC_CACHE


def _prep_xt(x):
    # [S, D] -> X^T laid out [NSC, 128, NDC, SC] in bf16
    xt = x.T.astype(ml_dtypes.bfloat16)                 # [D, S]
    return np.ascontiguousarray(
        xt.reshape(NDC, 128, NSC, SC).transpose(2, 1, 0, 3)
    )


def _prep_w(w):
    # [1024, GD] -> [128, NDC, GD] bf16
    return np.ascontiguousarray(
        w.astype(ml_dtypes.bfloat16).reshape(NDC, 128, GD).transpose(1, 0, 2))


def _prep_w_qk(w):
    # [1024, GD] -> c-major [128, 2, NDC, 128] bf16
    return np.ascontiguousarray(
        w.astype(ml_dtypes.bfloat16).reshape(NDC, 128, 2, 128)
        .transpose(1, 2, 0, 3))


def _prep_wo(w):
    # [GD, 1024] -> [128, 2, 1024] bf16
    return np.ascontiguousarray(
        w.astype(ml_dtypes.bfloat16).reshape(2, 128, D).transpose(1, 0, 2))


def kernel(q, k, v, Wq, bq, Wk, bk, Wv, bv, Wo, bo):
    q = np.asarray(q, np.float32)
    k = np.asarray(k, np.float32)
    v = np.asarray(v, np.float32)
    Wq = np.asarray(Wq, np.float32)
    Wk = np.asarray(Wk, np.float32)
    Wv = np.asarray(Wv, np.float32)
    Wo = np.asarray(Wo, np.float32)
    bq = np.asarray(bq, np.float32)
    bk = np.asarray(bk, np.float32)
    bv = np.asarray(bv, np.float32)
    bo = np.asarray(bo, np.float32)

    nc = _get_nc()

    xqt = [_prep_xt(q[b]) for b in range(B)]
    xkt = [_prep_xt(k[b]) for b in range(B)]
    xvt = [_prep_xt(v[b]) for b in range(B)]
    idn = np.eye(128, dtype=ml_dtypes.bfloat16)

    in_maps = []
    for core in range(8):
        b, g = divmod(core, 4)
        gs = slice(g * GD, (g + 1) * GD)
        in_maps.append({
            "xqt": xqt[b], "xkt": xkt[b], "xvt": xvt[b],
            "wq": _prep_w_qk(Wq[:, gs] * SCALE),
            "wk": _prep_w_qk(Wk[:, gs]),
            "wv": _prep_w(Wv[:, gs]),
            "wo": _prep_wo(Wo[gs, :]),
            "bq": np.ascontiguousarray(bq[gs] * SCALE),
            "bk": np.ascontiguousarray(bk[gs]),
            "bv": np.ascontiguousarray(bv[gs]),
            "idn": idn,
        })

    res = run_bass_kernel_spmd(nc, in_maps, core_ids=list(range(8)))

    out = np.empty((B, S, D), np.float32)
    for b in range(B):
        acc = np.asarray(res.results[4 * b]["out"], np.float32).copy()
        for g in range(1, 4):
            acc += np.asarray(res.results[4 * b + g]["out"], np.float32)
        out[b] = acc + bo
    return out

